# revision 1
# baseline (speedup 1.0000x reference)
"""E8 quantizer v6: 4 DVE reduces, packed argmax, PE y-assembly, lean smalls.

y = f1 + sg*(oh*gq + c01/2);  oh = (pk == (c01 ? mn : mx)) ungated;
gq = (1-2*c01)*poc = po1 - c01*(po1+po2)  (parity gate in multiplier).
PE: u3 = I*f1b + (-64I)*sg ; yp = I*f1b + I*Cv  (bf16, PSUM halves).
"""
import numpy as np
import concourse.bass as bass
import concourse.mybir as mybir
from concourse.tile import TileContext

AL = mybir.AluOpType
AF = mybir.ActivationFunctionType
AX = mybir.AxisListType
F32 = mybir.dt.float32
U8 = mybir.dt.uint8
I32 = mybir.dt.int32
BF16 = mybir.dt.bfloat16
MAGIC = float(np.float32(12582912.0))

N_ROWS_FULL = 8388608
DIM = 8
NCORES = 8
ROWS = N_ROWS_FULL // NCORES
F = 1024


def _split_multiwaits(nc):
    n = 0
    for f in nc.m.functions:
        for bb in f.blocks:
            newlist = []
            for ins in bb.instructions:
                si = getattr(ins, "sync_info", None)
                if si is not None and si.on_wait is not None and len(si.on_wait) > 1:
                    waits = list(si.on_wait)
                    for w in waits[:-1]:
                        nop = mybir.InstNoOp(name=f"I-mwfix-{n}", ins=[], outs=[])
                        n += 1
                        nop.engine = ins.engine
                        nop.sync_info = mybir.SyncInfo(on_wait=[w], on_update=[])
                        newlist.append(nop)
                    si.on_wait = [waits[-1]]
                newlist.append(ins)
            bb.instructions = newlist
    return n


BITVEC_OPS = (AL.logical_shift_left, AL.logical_shift_right,
              AL.arith_shift_left, AL.arith_shift_right, AL.bitwise_and,
              AL.bitwise_or, AL.bitwise_xor)


def _fix_bitvec_imms(nc):
    """walrus requires integer ImmVal (matching src/dst dtype) for bitvec
    alu ops; bass lowers python ints as f32 imms in scalar_tensor_tensor."""
    for fn in nc.m.functions:
        for bb in fn.blocks:
            for ins in bb.instructions:
                if not isinstance(ins, mybir.InstTensorScalarPtr):
                    continue
                ops = [getattr(ins, "op0", None), getattr(ins, "op1", None)]
                if not any(o in BITVEC_OPS for o in ops if o is not None):
                    continue
                new_ins = []
                for a in ins.ins:
                    if isinstance(a, mybir.ImmediateValue) and \
                            a.dtype == mybir.dt.float32:
                        a = mybir.ImmediateValue(dtype=mybir.dt.int32,
                                                 value=int(a.value))
                    new_ins.append(a)
                ins.ins = new_ins


def _g3(ap):
    return ap.rearrange("p (r c) -> p r c", c=8)


def _bc(ap_2d):
    p, r = ap_2d.shape
    return ap_2d.unsqueeze(2).broadcast_to((p, r, 8))


def build_nc(rows=ROWS, f=F, num_devices=NCORES, fix_multiwaits=True):
    elems = rows * DIM
    assert elems % (128 * f) == 0
    ntiles = elems // (128 * f)
    R = f // 8

    nc = bass.Bass("TRN2", num_devices=num_devices, debug=False)
    x = nc.dram_tensor("x", [rows, DIM], F32, kind="ExternalInput")
    y = nc.dram_tensor("y", [rows, DIM], F32, kind="ExternalOutput")
    xt = x[:].flatten().rearrange("(t p f) -> t p f", p=128, f=f)
    yt = y[:].flatten().rearrange("(t p f) -> t p f", p=128, f=f)

    with TileContext(nc) as tc:
        with tc.tile_pool(name="cst", bufs=1) as cst, \
             tc.tile_pool(name="ioi", bufs=4) as ioi, \
             tc.tile_pool(name="ioo", bufs=4) as ioo, \
             tc.tile_pool(name="pA", bufs=2) as pA, \
             tc.tile_pool(name="pB", bufs=3) as pB, \
             tc.tile_pool(name="pC", bufs=4) as pC, \
             tc.tile_pool(name="pD", bufs=3) as pD, \
             tc.tile_pool(name="grs", bufs=3) as grs, \
             tc.tile_pool(name="grl", bufs=4) as grl, \
             tc.tile_pool(name="ps", bufs=2, space="PSUM") as ps, \
             tc.tile_pool(name="ps2", bufs=2, space="PSUM") as ps2:

            it = cst.tile([128, f], I32)
            nc.gpsimd.iota(it[:], pattern=[[1, f]], base=0, channel_multiplier=0)
            i8 = cst.tile([128, f], I32, tag="i8")
            nc.vector.tensor_scalar(i8[:], it[:], 7, None, AL.bitwise_and)
            ip = cst.tile([128, 128], I32, tag="ip")
            nc.gpsimd.iota(ip[:], pattern=[[0, 128]], base=0,
                           channel_multiplier=1)
            jp = cst.tile([128, 128], I32, tag="jp")
            nc.gpsimd.iota(jp[:], pattern=[[1, 128]], base=0,
                           channel_multiplier=0)
            W1 = cst.tile([128, 128], BF16, tag="W1")
            nc.vector.tensor_tensor(W1[:], ip[:], jp[:], AL.is_equal)
            Wm64 = cst.tile([128, 128], BF16, tag="Wm64")
            nc.vector.tensor_scalar(Wm64[:], W1[:], -64.0, None, AL.mult)
            c128 = cst.tile([128, R], F32, tag="c128")
            nc.gpsimd.memset(c128[:], 128.0)
            chf = cst.tile([128, R], F32, tag="chf")
            nc.gpsimd.memset(chf[:], 0.5)

            for t in range(ntiles):
                xv = ioi.tile([128, f], F32, tag="xv")
                nc.sync.dma_start(xv[:], xt[t])

                # rounds / residual / sign
                t1 = pA.tile([128, f], F32, tag="t1")
                nc.scalar.activation(t1[:], xv[:], AF.Copy, bias=MAGIC)
                f1b = pC.tile([128, f], BF16, tag="f1b")
                nc.scalar.activation(f1b[:], t1[:], AF.Copy, bias=-MAGIC)
                d1 = pB.tile([128, f], F32, tag="d1")
                nc.gpsimd.tensor_tensor(d1[:], xv[:], f1b[:], AL.subtract)
                sg = pC.tile([128, f], BF16, tag="sg")
                nc.scalar.activation(sg[:], d1[:], AF.Sign)

                # packed |d|+idx ; u3 parity carrier on PE
                ad1 = pB.tile([128, f], F32, tag="ad1")
                nc.gpsimd.tensor_tensor(ad1[:], d1[:], sg[:], AL.mult)
                pk = pD.tile([128, f], I32, tag="pk")
                nc.gpsimd.tensor_tensor(pk[:], ad1[:].bitcast(I32), i8[:],
                                        AL.add)
                u3 = ps2.tile([128, f], F32, tag="u3")
                for h in range(f // 512):
                    hs = slice(512 * h, 512 * (h + 1))
                    nc.tensor.matmul(u3[:, hs], W1[:], f1b[:, hs],
                                     start=True, stop=False)
                    nc.tensor.matmul(u3[:, hs], Wm64[:], sg[:, hs],
                                     start=False, stop=True)

                # reduces
                L1 = grs.tile([128, R], F32, tag="L1")
                nc.vector.tensor_reduce(L1[:], _g3(d1[:]), AX.X, AL.add,
                                        apply_absolute_value=True)
                mx = grl.tile([128, R], F32, tag="mx")
                nc.vector.tensor_reduce(mx[:], _g3(pk[:].bitcast(F32)), AX.X,
                                        AL.max)
                mn = grl.tile([128, R], F32, tag="mn")
                nc.vector.tensor_reduce(mn[:], _g3(pk[:].bitcast(F32)), AX.X,
                                        AL.min)
                U3 = grs.tile([128, R], F32, tag="U3")
                nc.vector.tensor_reduce(U3[:], _g3(u3[:]), AX.X, AL.add)

                # parities: U3 = S1 + 128*nneg - 512
                nnM = grs.tile([128, R], F32, tag="nnM")
                nc.scalar.activation(nnM[:], U3[:], AF.Copy, bias=MAGIC + 4.0,
                                     scale=1.0 / 128.0)
                nn = grs.tile([128, R], F32, tag="nn")
                nc.scalar.activation(nn[:], nnM[:], AF.Copy, bias=-MAGIC)
                nn128 = grs.tile([128, R], F32, tag="nn128")
                nc.gpsimd.tensor_tensor(nn128[:], nn[:], c128[:], AL.mult)
                s1v = grs.tile([128, R], F32, tag="s1v")
                nc.gpsimd.tensor_tensor(s1v[:], U3[:], nn128[:], AL.subtract)
                h1 = grs.tile([128, R], F32, tag="h1")
                nc.scalar.activation(h1[:], s1v[:], AF.Copy, bias=MAGIC,
                                     scale=0.5)
                h2 = grs.tile([128, R], F32, tag="h2")
                nc.scalar.activation(h2[:], h1[:], AF.Copy, bias=-2.0 * MAGIC,
                                     scale=2.0)
                po1 = grs.tile([128, R], F32, tag="po1")
                nc.vector.tensor_tensor(po1[:], s1v[:], h2[:], AL.not_equal)
                hn1 = grs.tile([128, R], F32, tag="hn1")
                nc.vector.tensor_scalar(hn1[:], nn[:], 0.5, MAGIC, AL.mult,
                                        AL.add)
                hn2 = grs.tile([128, R], F32, tag="hn2")
                nc.vector.tensor_scalar(hn2[:], hn1[:], 2.0, -2.0 * MAGIC,
                                        AL.mult, AL.add)
                pnn = grs.tile([128, R], F32, tag="pnn")
                nc.vector.tensor_tensor(pnn[:], nn[:], hn2[:], AL.not_equal)
                po2 = grs.tile([128, R], F32, tag="po2")
                nc.vector.tensor_tensor(po2[:], po1[:], pnn[:], AL.not_equal)

                # m values / q-compare
                tm1 = grs.tile([128, R], F32, tag="tm1")
                nc.vector.tensor_scalar(tm1[:], mx[:], -2.0, 1.0,
                                        AL.mult, AL.add)
                tm2 = grs.tile([128, R], F32, tag="tm2")
                nc.vector.tensor_scalar(tm2[:], mn[:], 2.0, None,
                                        AL.mult)
                A2s = grs.tile([128, R], F32, tag="A2s")
                nc.gpsimd.tensor_tensor(A2s[:], po1[:], tm1[:], AL.mult)
                A1s = grs.tile([128, R], F32, tag="A1s")
                nc.gpsimd.tensor_tensor(A1s[:], po2[:], tm2[:], AL.mult)
                D1s = grs.tile([128, R], F32, tag="D1s")
                nc.scalar.activation(D1s[:], L1[:], AF.Copy, bias=2.0,
                                     scale=-1.0)
                D2s = grs.tile([128, R], F32, tag="D2s")
                nc.gpsimd.tensor_tensor(D2s[:], D1s[:], A1s[:], AL.add)
                c01 = grl.tile([128, R], F32, tag="c01")
                nc.vector.tensor_tensor(c01[:], D2s[:], A2s[:], AL.is_lt)

                # select + parity-in-multiplier
                c01u = grl.tile([128, R], U8, tag="c01u")
                nc.vector.tensor_tensor(c01u[:], D2s[:], A2s[:], AL.is_lt)
                mcc = grl.tile([128, R], F32, tag="mcc")
                nc.vector.tensor_scalar(mcc[:], mx[:], 0.0, None, AL.add)
                nc.vector.copy_predicated(mcc[:], c01u[:], mn[:])
                v1s = grs.tile([128, R], F32, tag="v1s")  # po1+po2
                nc.gpsimd.tensor_tensor(v1s[:], po1[:], po2[:], AL.add)
                v2s = grs.tile([128, R], F32, tag="v2s")  # c01*(po1+po2)
                nc.gpsimd.tensor_tensor(v2s[:], c01[:], v1s[:], AL.mult)
                gq = grl.tile([128, R], F32, tag="gq")  # po1 - c01*(po1+po2)
                nc.gpsimd.tensor_tensor(gq[:], po1[:], v2s[:], AL.subtract)
                ch = grl.tile([128, R], F32, tag="ch")  # c01/2
                nc.gpsimd.tensor_tensor(ch[:], c01[:], chf[:], AL.mult)

                # one-hot + output
                dfv = pD.tile([128, f], F32, tag="dfv")
                nc.gpsimd.tensor_tensor(_g3(dfv[:]), _g3(pk[:].bitcast(F32)),
                                        _bc(mcc[:]), AL.subtract)
                oh = pD.tile([128, f], F32, tag="oh")
                nc.vector.tensor_scalar(oh[:], dfv[:], 0.0, None, AL.is_equal)
                Av = pD.tile([128, f], F32, tag="Av")
                nc.gpsimd.tensor_tensor(_g3(Av[:]), _g3(oh[:]), _bc(gq[:]),
                                        AL.mult)
                Bv = pD.tile([128, f], F32, tag="Bv")
                nc.gpsimd.tensor_tensor(_g3(Bv[:]), _g3(Av[:]), _bc(ch[:]),
                                        AL.add)
                Cv = pC.tile([128, f], BF16, tag="Cv")
                nc.gpsimd.tensor_tensor(Cv[:], sg[:], Bv[:], AL.mult)
                yp = ps.tile([128, f], F32, tag="yp")
                for h in range(f // 512):
                    hs = slice(512 * h, 512 * (h + 1))
                    nc.tensor.matmul(yp[:, hs], W1[:], f1b[:, hs],
                                     start=True, stop=False)
                    nc.tensor.matmul(yp[:, hs], W1[:], Cv[:, hs],
                                     start=False, stop=True)
                yv = ioo.tile([128, f], F32, tag="yv")
                nc.scalar.activation(yv[:], yp[:], AF.Copy)
                nc.sync.dma_start(yt[t], yv[:])

    _fix_bitvec_imms(nc)
    if fix_multiwaits:
        _split_multiwaits(nc)
    return nc


def reference_numpy_v6(x):
    x = np.asarray(x, dtype=np.float32)
    n = x.shape[0]
    t1 = (x + np.float32(MAGIC)).astype(np.float32)
    f1 = (t1 - np.float32(MAGIC)).astype(np.float32)
    d1 = (x - f1).astype(np.float32)
    sg = np.sign(d1).astype(np.float32)
    ad1 = (d1 * sg).astype(np.float32)
    pk = ad1.view(np.int32) + np.tile(np.arange(8, dtype=np.int32), (n, 1))
    pkf = pk.view(np.float32)
    mx = pkf.max(axis=1)
    mn = pkf.min(axis=1)
    a = np.abs(d1).astype(np.float32)
    L1 = a[:, 0]
    for j in range(1, 8):
        L1 = (L1 + a[:, j]).astype(np.float32)
    u3 = (f1 - np.float32(64.0) * sg).astype(np.float32)
    U3 = u3[:, 0]
    for j in range(1, 8):
        U3 = (U3 + u3[:, j]).astype(np.float32)
    nneg = np.rint(U3 / 128.0 + 4.0).astype(np.int64)
    S1 = np.rint(U3 - 128.0 * nneg + 512.0).astype(np.int64)
    po1 = (S1 & 1).astype(np.float32)
    po2 = ((S1 ^ nneg) & 1).astype(np.float32)
    tm1 = (np.float32(1.0) - np.float32(2.0) * mx).astype(np.float32)
    tm2 = (np.float32(2.0) * mn).astype(np.float32)
    D = (np.float32(2.0) - L1) + po2 * tm2 - po1 * tm1
    c01 = (D < 0).astype(np.float32)
    mcc = np.where(c01 > 0, mn, mx)
    gq = po1 - c01 * (po1 + po2)
    oh = (pkf == mcc[:, None]).astype(np.float32)
    z = oh * gq[:, None] + (c01 * np.float32(0.5))[:, None]
    yv = (f1 + sg * z).astype(np.float32)
    return yv



_NC_CACHE = {}


def _get_nc(rows, f):
    key = (rows, f)
    if key not in _NC_CACHE:
        _NC_CACHE[key] = build_nc(rows, f)
    return _NC_CACHE[key]


def kernel(x: np.ndarray, _trace=False) -> np.ndarray:
    from concourse.bass_utils import run_bass_kernel_spmd
    assert x.shape == (N_ROWS_FULL, DIM), x.shape
    x = np.ascontiguousarray(np.asarray(x, dtype=np.float32))
    nc = _get_nc(ROWS, F)
    in_maps = [
        {"x": np.ascontiguousarray(x[i * ROWS:(i + 1) * ROWS])}
        for i in range(NCORES)
    ]
    res = run_bass_kernel_spmd(nc, in_maps, core_ids=list(range(NCORES)),
                               trace=_trace)
    out = np.empty_like(x)
    for i in range(NCORES):
        out[i * ROWS:(i + 1) * ROWS] = res.results[i]["y"]
    return out



# revision 34
# speedup vs baseline: 1.3127x; 1.3127x over previous
"""E8 quantizer v13: HW-legal engine-rebalanced pipeline.

Per [128,1024] tile (G=4 tiles per batched-smalls group), 3-stage
software pipeline A(g) / B(g-1) / C(g-2):
- round via f32 magic on Act (t1 = x+M, f1b = t1-M -> bf16, exact).
- d1 = x - f1 on Pool (tensor_tensor; Pool ISA = add/sub/mult only).
- sgb = Sign(d1) on Act (input has no exact-integer x, so no zeros).
- pk = |d1| via bitwise-and on DVE.
- L1 = add-tree over pk on Pool; mx/mn = abs-reduces over d1 on DVE.
- parity sums Uf = sum(f1), Us = sum(sg) on PE (8 identity matmuls each
  into PSUM); parities = bit0 of (sum + MAGIC) floats, decoded on DVE.
- per-row smalls batched [128, 512]; mcc select via exact mults
  (mn*c01 + mx*(1-c01): one addend always zero).
- one-hot: dfv = pk - mcc (Pool), oh = (dfv == 0) (DVE ts); assembly
  Av = oh*gq + ch in-place on Pool (bc along the c-major layout),
  Cv = Av*sgb on DVE (packed bf16 2x); y = f1b + Cv on PE into PSUM;
  Act evacuates as bf16; DMA out bf16 (exact: y half-integers < 256).
"""
import numpy as np
import concourse.bass as bass
import concourse.mybir as mybir
from concourse.tile import TileContext

AL = mybir.AluOpType
AF = mybir.ActivationFunctionType
AX = mybir.AxisListType
F32 = mybir.dt.float32
U8 = mybir.dt.uint8
U16 = mybir.dt.uint16
I32 = mybir.dt.int32
BF16 = mybir.dt.bfloat16
MAGIC = float(np.float32(12582912.0))
BIGF = 1.0e30

N_ROWS_FULL = 8388608
DIM = 8
NCORES = 8
ROWS = N_ROWS_FULL // NCORES
F = 1024
G = 4


def _split_multiwaits(nc):
    n = 0
    for f in nc.m.functions:
        for bb in f.blocks:
            newlist = []
            for ins in bb.instructions:
                si = getattr(ins, "sync_info", None)
                if si is not None and si.on_wait is not None and len(si.on_wait) > 1:
                    waits = list(si.on_wait)
                    for w in waits[:-1]:
                        nop = mybir.InstNoOp(name=f"I-mwfix-{n}", ins=[], outs=[])
                        n += 1
                        nop.engine = ins.engine
                        nop.sync_info = mybir.SyncInfo(on_wait=[w], on_update=[])
                        newlist.append(nop)
                    si.on_wait = [waits[-1]]
                newlist.append(ins)
            bb.instructions = newlist
    return n


BITVEC_OPS = (AL.logical_shift_left, AL.logical_shift_right,
              AL.arith_shift_left, AL.arith_shift_right, AL.bitwise_and,
              AL.bitwise_or, AL.bitwise_xor)


def _fix_bitvec_imms(nc):
    """walrus requires integer ImmVal (matching src/dst dtype) for bitvec
    alu ops; bass lowers python ints as f32 imms in scalar_tensor_tensor."""
    for fn in nc.m.functions:
        for bb in fn.blocks:
            for ins in bb.instructions:
                if not isinstance(ins, mybir.InstTensorScalarPtr):
                    continue
                ops = [getattr(ins, "op0", None), getattr(ins, "op1", None)]
                if not any(o in BITVEC_OPS for o in ops if o is not None):
                    continue
                new_ins = []
                for a in ins.ins:
                    if isinstance(a, mybir.ImmediateValue) and \
                            a.dtype == mybir.dt.float32:
                        a = mybir.ImmediateValue(dtype=mybir.dt.int32,
                                                 value=int(a.value))
                    new_ins.append(a)
                ins.ins = new_ins


def build_nc(rows=ROWS, f=F, num_devices=NCORES, fix_multiwaits=True):
    elems = rows * DIM
    assert elems % (128 * f) == 0
    ntiles = elems // (128 * f)
    assert ntiles % G == 0
    ngroups = ntiles // G
    R = f // 8
    GR = G * R

    nc = bass.Bass("TRN2", num_devices=num_devices, debug=False)
    x = nc.dram_tensor("x", [rows, DIM], F32, kind="ExternalInput")
    y = nc.dram_tensor("y", [rows, DIM], BF16, kind="ExternalOutput")
    xt = x[:].flatten().rearrange("(t p f) -> t p f", p=128, f=f)
    yt = y[:].flatten().rearrange("(t p f) -> t p f", p=128, f=f)

    def dv(ap):  # [p, c, r] view of c-major storage (packed r)
        return ap.rearrange("p (c r) -> p c r", c=8)

    def rv(ap):  # [p, r, c] view of c-major storage (c stride R)
        return ap.rearrange("p (c r) -> p r c", c=8)

    def bcr(ap_2d):  # [128, R] -> [128, 8, R] broadcast over c
        p, r = ap_2d.shape
        return ap_2d.unsqueeze(1).broadcast_to((p, 8, r))

    with TileContext(nc) as tc:
        with tc.tile_pool(name="cst", bufs=1) as cst, \
             tc.tile_pool(name="ioi", bufs=4) as ioi, \
             tc.tile_pool(name="pnd", bufs=2) as pnd, \
             tc.tile_pool(name="spk", bufs=3) as spk, \
             tc.tile_pool(name="sf1", bufs=3) as sf1, \
             tc.tile_pool(name="ssg", bufs=3) as ssg, \
             tc.tile_pool(name="trp", bufs=1) as trp, \
             tc.tile_pool(name="red", bufs=2) as red, \
             tc.tile_pool(name="smt", bufs=1) as smt, \
             tc.tile_pool(name="smo", bufs=3) as smo, \
             tc.tile_pool(name="asm", bufs=2) as asm, \
             tc.tile_pool(name="yvp", bufs=2) as yvp, \
             tc.tile_pool(name="psU", bufs=2, space="PSUM") as psU, \
             tc.tile_pool(name="psV", bufs=2, space="PSUM") as psV, \
             tc.tile_pool(name="psY", bufs=2, space="PSUM") as psY:

            ip = cst.tile([128, 128], I32, tag="ip")
            nc.gpsimd.iota(ip[:], pattern=[[0, 128]], base=0,
                           channel_multiplier=1)
            jp = cst.tile([128, 128], I32, tag="jp")
            nc.gpsimd.iota(jp[:], pattern=[[1, 128]], base=0,
                           channel_multiplier=0)
            ident = cst.tile([128, 128], BF16, tag="ident")
            nc.vector.tensor_tensor(ident[:], ip[:], jp[:], AL.is_equal)

            def phase_in(ctx):
                g = ctx["g"]
                xvs = []
                for t in range(G):
                    xv = ioi.tile([128, f], F32, tag="xv")
                    nc.sync.dma_start(xv[:], xt[g * G + t])
                    xvs.append(xv)
                ctx["xvs"] = xvs

            def phase_a(g):
                ctx = {}
                ctx["g"] = g
                pks = ctx["pks"] = spk.tile([128, G * f], F32, tag="pks", name="pks")
                f1s = ctx["f1s"] = sf1.tile([128, G * f], BF16, tag="f1s", name="f1s")
                sgs = ctx["sgs"] = ssg.tile([128, G * f], BF16, tag="sgs", name="sgs")
                L1s = ctx["L1s"] = red.tile([128, GR], F32, tag="L1s", name="L1s")
                mxs = ctx["mxs"] = red.tile([128, GR], F32, tag="mxs", name="mxs")
                mns = ctx["mns"] = red.tile([128, GR], F32, tag="mns", name="mns")
                Uf = ctx["Uf"] = psU.tile([128, GR], F32, tag="Uf", name="Uf")
                Us = ctx["Us"] = psV.tile([128, GR], F32, tag="Us", name="Us")

                return ctx

            def phase_a_tile(ctx, t):
                    g = ctx["g"]
                    pks, f1s, sgs = ctx["pks"], ctx["f1s"], ctx["sgs"]
                    L1s, mxs, mns = ctx["L1s"], ctx["mxs"], ctx["mns"]
                    Uf, Us = ctx["Uf"], ctx["Us"]
                    gt = g * G + t
                    ts_ = slice(t * f, (t + 1) * f)
                    rs = slice(t * R, (t + 1) * R)
                    xv = ctx["xvs"][t]
                    xv_cr = xv[:].rearrange("p (r c) -> p c r", c=8)

                    t1 = pnd.tile([128, f], F32, tag="t1")
                    nc.scalar.activation(dv(t1[:]), xv_cr, AF.Copy,
                                         bias=MAGIC)
                    f1b = f1s[:, ts_]
                    nc.scalar.activation(f1b, t1[:], AF.Copy, bias=-MAGIC)

                    # d1 = x - f1 (c-major storage)
                    d1 = pnd.tile([128, f], F32, tag="d1")
                    nc.gpsimd.tensor_tensor(dv(d1[:]), xv_cr, dv(f1b),
                                            AL.subtract)

                    sgb = sgs[:, ts_]
                    nc.scalar.activation(sgb, d1[:], AF.Sign)

                    pk = pks[:, ts_]
                    nc.vector.tensor_scalar(pk.bitcast(I32),
                                            d1[:].bitcast(I32),
                                            0x7fffffff, None, AL.bitwise_and)

                    # mx/mn: abs-reduces over the component axis
                    nc.vector.tensor_reduce(mxs[:, rs], rv(d1[:]), AX.X,
                                            AL.max,
                                            apply_absolute_value=True)
                    nc.vector.tensor_reduce(mns[:, rs], rv(d1[:]), AX.X,
                                            AL.min,
                                            apply_absolute_value=True)

                    # L1 via Pool add-tree on pk = |d|
                    pk3 = dv(pk)
                    s1 = trp.tile([128, f // 2], F32, tag="s1")
                    s1v = s1[:].rearrange("p (c r) -> p c r", c=4)
                    nc.gpsimd.tensor_tensor(s1v, pk3[:, 0:4], pk3[:, 4:8],
                                            AL.add)
                    s2 = trp.tile([128, f // 4], F32, tag="s2")
                    s2v = s2[:].rearrange("p (c r) -> p c r", c=2)
                    nc.gpsimd.tensor_tensor(s2v, s1v[:, 0:2], s1v[:, 2:4],
                                            AL.add)
                    nc.gpsimd.tensor_tensor(L1s[:, rs], s2v[:, 0],
                                            s2v[:, 1], AL.add)

                    for c in range(8):
                        nc.tensor.matmul(Uf[:, rs], ident[:],
                                         f1b[:, c * R:(c + 1) * R],
                                         start=(c == 0), stop=(c == 7))
                    for c in range(8):
                        nc.tensor.matmul(Us[:, rs], ident[:],
                                         sgb[:, c * R:(c + 1) * R],
                                         start=(c == 0), stop=(c == 7))

            def phase_b(ctx):
                Uf, Us = ctx["Uf"], ctx["Us"]
                L1s, mxs, mns = ctx["L1s"], ctx["mxs"], ctx["mns"]
                fM = smt.tile([128, GR], F32, tag="fM")
                nc.scalar.activation(fM[:], Uf[:], AF.Copy, bias=MAGIC)
                ngM = smt.tile([128, GR], F32, tag="ngM")
                nc.scalar.activation(ngM[:], Us[:], AF.Copy,
                                     bias=MAGIC + 4.0, scale=-0.5)
                po1i = smt.tile([128, GR], I32, tag="po1i")
                nc.vector.tensor_scalar(po1i[:], fM[:].bitcast(I32), 1, None,
                                        AL.bitwise_and)
                pnni = smt.tile([128, GR], I32, tag="pnni")
                nc.vector.tensor_scalar(pnni[:], ngM[:].bitcast(I32), 1, None,
                                        AL.bitwise_and)
                po1b = smt.tile([128, GR], BF16, tag="po1b")
                nc.vector.tensor_copy(po1b[:], po1i[:])
                pnnb = smt.tile([128, GR], BF16, tag="pnnb")
                nc.vector.tensor_copy(pnnb[:], pnni[:])
                po2b = smt.tile([128, GR], BF16, tag="po2b")
                nc.vector.tensor_tensor(po2b[:], po1b[:], pnnb[:],
                                        AL.not_equal)
                tm1 = smt.tile([128, GR], F32, tag="tm1")
                nc.vector.tensor_scalar(tm1[:], mxs[:], -2.0, 1.0,
                                        AL.mult, AL.add)
                tm2 = smt.tile([128, GR], F32, tag="tm2")
                nc.vector.tensor_scalar(tm2[:], mns[:], 2.0, None, AL.mult)
                D1s = smt.tile([128, GR], F32, tag="D1s")
                nc.vector.tensor_scalar(D1s[:], L1s[:], -1.0, 2.0,
                                        AL.mult, AL.add)
                A2s = smt.tile([128, GR], F32, tag="A2s")
                nc.gpsimd.tensor_tensor(A2s[:], po1b[:], tm1[:], AL.mult)
                A1s = smt.tile([128, GR], F32, tag="A1s")
                nc.gpsimd.tensor_tensor(A1s[:], po2b[:], tm2[:], AL.mult)
                e1s = smt.tile([128, GR], F32, tag="e1s")
                nc.gpsimd.tensor_tensor(e1s[:], D1s[:], A2s[:], AL.subtract)
                dfc = smt.tile([128, GR], F32, tag="dfc")
                nc.gpsimd.tensor_tensor(dfc[:], e1s[:], A1s[:], AL.add)
                c01 = smt.tile([128, GR], F32, tag="c01")
                nc.vector.tensor_scalar(c01[:], dfc[:], 0.0, None, AL.is_lt)
                nc01 = smt.tile([128, GR], F32, tag="nc01")
                nc.vector.tensor_scalar(nc01[:], c01[:], -1.0, 1.0,
                                        AL.mult, AL.add)
                # mcc = mn*c01 + mx*(1-c01): exact (one addend always 0)
                mA = smt.tile([128, GR], F32, tag="mA")
                nc.gpsimd.tensor_tensor(mA[:], mns[:], c01[:], AL.mult)
                mB = smt.tile([128, GR], F32, tag="mB")
                nc.gpsimd.tensor_tensor(mB[:], mxs[:], nc01[:], AL.mult)
                mcc = smo.tile([128, GR], F32, tag="mcc")
                nc.gpsimd.tensor_tensor(mcc[:], mA[:], mB[:], AL.add)
                v1s = smt.tile([128, GR], BF16, tag="v1s")
                nc.vector.tensor_tensor(v1s[:], po1b[:], po2b[:], AL.add)
                w2b = smt.tile([128, GR], BF16, tag="w2b")
                nc.gpsimd.tensor_tensor(w2b[:], c01[:], v1s[:], AL.mult)
                gq = smo.tile([128, GR], BF16, tag="gq")
                nc.gpsimd.tensor_tensor(gq[:], po1b[:], w2b[:], AL.subtract)
                ch = smo.tile([128, GR], BF16, tag="ch")
                nc.vector.tensor_scalar(ch[:], c01[:], 0.5, None, AL.mult)
                ctx["mcc"], ctx["gq"], ctx["ch"] = mcc, gq, ch

            def phase_c_oh(ctx, t):
                    pks = ctx["pks"]
                    mcc = ctx["mcc"]
                    ts_ = slice(t * f, (t + 1) * f)
                    rs = slice(t * R, (t + 1) * R)
                    pkf3 = dv(pks[:, ts_])
                    dfv = pnd.tile([128, f], F32, tag="t1")
                    nc.gpsimd.tensor_tensor(dv(dfv[:]), pkf3,
                                            bcr(mcc[:, rs]), AL.subtract)
                    oh = asm.tile([128, f], BF16, tag="oh", bufs=2)
                    nc.vector.tensor_scalar(oh[:], dfv[:], 0.0, None,
                                            AL.is_equal)
                    ctx.setdefault("ohs", []).append(oh)

            def phase_c_tile(ctx, t):
                    pks, f1s, sgs = ctx["pks"], ctx["f1s"], ctx["sgs"]
                    mcc, gq, ch = ctx["mcc"], ctx["gq"], ctx["ch"]
                    gt = ctx["g"] * G + t
                    ts_ = slice(t * f, (t + 1) * f)
                    rs = slice(t * R, (t + 1) * R)
                    oh = ctx["ohs"][t]
                    Av = asm.tile([128, f], BF16, tag="Av")
                    nc.gpsimd.tensor_tensor(dv(Av[:]), dv(oh[:]),
                                            bcr(gq[:, rs]), AL.mult)
                    nc.gpsimd.tensor_tensor(dv(Av[:]), dv(Av[:]),
                                            bcr(ch[:, rs]), AL.add)
                    Cv = Av
                    nc.vector.tensor_tensor(Cv[:], Cv[:], sgs[:, ts_],
                                            AL.mult)
                    yp = psY.tile([128, f], F32, tag="yp")
                    f1iv = rv(f1s[:, ts_])
                    cviv = rv(Cv[:])
                    h = R // 2
                    for hh in range(2):
                        msl = slice(hh * h, (hh + 1) * h)
                        osl = slice(hh * (f // 2), (hh + 1) * (f // 2))
                        nc.tensor.matmul(yp[:, osl], ident[:], f1iv[:, msl],
                                         start=True, stop=False)
                        nc.tensor.matmul(yp[:, osl], ident[:], cviv[:, msl],
                                         start=False, stop=True)
                    yv = yvp.tile([128, f], BF16, tag="yv")
                    nc.scalar.activation(yv[:], yp[:], AF.Copy)
                    nc.sync.dma_start(yt[gt], yv[:])

            # software pipeline: B/C of group g-1 run alongside A of group g
            p1 = None  # group awaiting B
            p2 = None  # group awaiting C
            nxt = phase_a(0)
            phase_in(nxt)

            def do_b(ctx):
                phase_b(ctx)
                for t in range(G):
                    phase_c_oh(ctx, t)

            def do_c(ctx):
                for t in range(G):
                    phase_c_tile(ctx, t)

            for g in range(ngroups):
                cur = nxt
                if g + 1 < ngroups:
                    nxt = phase_a(g + 1)
                    phase_in(nxt)
                for t in range(G):
                    phase_a_tile(cur, t)
                if p1 is not None:
                    do_b(p1)
                if p2 is not None:
                    do_c(p2)
                p2 = p1
                p1 = cur
            do_b(p1)
            do_c(p2)
            do_c(p1)

    _fix_bitvec_imms(nc)
    if fix_multiwaits:
        _split_multiwaits(nc)
    return nc


def reference_numpy_v8(x):
    """Bit-exact numpy mirror of the v13 kernel (row-logical space)."""
    x = np.asarray(x, dtype=np.float32)
    M = np.float32(MAGIC)
    t1 = (x + M).astype(np.float32)
    f1 = (t1 - M).astype(np.float32)
    d1 = (x - f1).astype(np.float32)
    sg = np.sign(d1).astype(np.float32)
    pkf = np.abs(d1).astype(np.float32)
    # L1 pair tree: ((a0+a4)+(a2+a6)) style
    s1 = (pkf[:, 0:4] + pkf[:, 4:8]).astype(np.float32)
    s2 = (s1[:, 0:2] + s1[:, 2:4]).astype(np.float32)
    L1 = (s2[:, 0] + s2[:, 1]).astype(np.float32)
    mx = pkf.max(axis=1)
    mn = pkf.min(axis=1)
    Uf = f1.sum(axis=1, dtype=np.float32)
    Us = sg.sum(axis=1, dtype=np.float32)
    fM = (Uf + M).astype(np.float32)
    ngM = (np.float32(-0.5) * Us + np.float32(MAGIC + 4.0)).astype(np.float32)
    po1 = (fM.view(np.int32) & 1).astype(np.float32)
    pnn = (ngM.view(np.int32) & 1).astype(np.float32)
    po2 = (po1 != pnn).astype(np.float32)
    tm1 = (np.float32(-2.0) * mx + np.float32(1.0)).astype(np.float32)
    tm2 = (np.float32(2.0) * mn).astype(np.float32)
    D1s = (np.float32(-1.0) * L1 + np.float32(2.0)).astype(np.float32)
    A2s = (po1 * tm1).astype(np.float32)
    A1s = (po2 * tm2).astype(np.float32)
    dfc = ((D1s - A2s).astype(np.float32) + A1s).astype(np.float32)
    c01 = (dfc < 0).astype(np.float32)
    mcc = np.where(c01 > 0, mn, mx)  # exact mult-select
    v1s = (po1 + po2).astype(np.float32)
    w2 = (c01 * v1s).astype(np.float32)
    gq = (po1 - w2).astype(np.float32)
    ch = (c01 * np.float32(0.5)).astype(np.float32)
    oh = (pkf == mcc[:, None]).astype(np.float32)
    Av = oh * gq[:, None]
    Bv = Av + ch[:, None]
    Cv = (sg * Bv).astype(np.float32)
    yv = (f1 + Cv).astype(np.float32)
    return yv


reference_numpy_v7 = reference_numpy_v8
reference_numpy_v6 = reference_numpy_v8


_NC_CACHE = {}


def _get_nc(rows, f):
    key = (rows, f)
    if key not in _NC_CACHE:
        _NC_CACHE[key] = build_nc(rows, f)
    return _NC_CACHE[key]


def kernel(x: np.ndarray, _trace=False) -> np.ndarray:
    from concourse.bass_utils import run_bass_kernel_spmd
    assert x.shape == (N_ROWS_FULL, DIM), x.shape
    x = np.ascontiguousarray(np.asarray(x, dtype=np.float32))
    nc = _get_nc(ROWS, F)
    in_maps = [
        {"x": np.ascontiguousarray(x[i * ROWS:(i + 1) * ROWS])}
        for i in range(NCORES)
    ]
    res = run_bass_kernel_spmd(nc, in_maps, core_ids=list(range(NCORES)),
                               trace=_trace)
    out = np.empty_like(x)
    for i in range(NCORES):
        out[i * ROWS:(i + 1) * ROWS] = \
            np.asarray(res.results[i]["y"]).astype(np.float32)
    return out


# revision 37
# speedup vs baseline: 1.3182x; 1.0041x over previous
"""E8 quantizer v13: HW-legal engine-rebalanced pipeline.

Per [128,1024] tile (G=4 tiles per batched-smalls group), 3-stage
software pipeline A(g) / B(g-1) / C(g-2):
- round via f32 magic on Act (t1 = x+M, f1b = t1-M -> bf16, exact).
- d1 = x - f1 on Pool (tensor_tensor; Pool ISA = add/sub/mult only).
- sgb = Sign(d1) on Act (input has no exact-integer x, so no zeros).
- pk = |d1| via bitwise-and on DVE.
- L1 = add-tree over pk on Pool; mx/mn = abs-reduces over d1 on DVE.
- parity sums Uf = sum(f1), Us = sum(sg) on PE (8 identity matmuls each
  into PSUM); parities = bit0 of (sum + MAGIC) floats, decoded on DVE.
- per-row smalls batched [128, 512]; mcc select via exact mults
  (mn*c01 + mx*(1-c01): one addend always zero).
- one-hot: dfv = pk - mcc (Pool), oh = (dfv == 0) (DVE ts); assembly
  Av = oh*gq + ch in-place on Pool (bc along the c-major layout),
  Cv = Av*sgb on DVE (packed bf16 2x); y = f1b + Cv on PE into PSUM;
  Act evacuates as bf16; DMA out bf16 (exact: y half-integers < 256).
"""
import numpy as np
import concourse.bass as bass
import concourse.mybir as mybir
from concourse.tile import TileContext

AL = mybir.AluOpType
AF = mybir.ActivationFunctionType
AX = mybir.AxisListType
F32 = mybir.dt.float32
U8 = mybir.dt.uint8
U16 = mybir.dt.uint16
I32 = mybir.dt.int32
BF16 = mybir.dt.bfloat16
MAGIC = float(np.float32(12582912.0))
BIGF = 1.0e30

N_ROWS_FULL = 8388608
DIM = 8
NCORES = 8
ROWS = N_ROWS_FULL // NCORES
F = 1024
G = 4


def _split_multiwaits(nc):
    n = 0
    for f in nc.m.functions:
        for bb in f.blocks:
            newlist = []
            for ins in bb.instructions:
                si = getattr(ins, "sync_info", None)
                if si is not None and si.on_wait is not None and len(si.on_wait) > 1:
                    waits = list(si.on_wait)
                    for w in waits[:-1]:
                        nop = mybir.InstNoOp(name=f"I-mwfix-{n}", ins=[], outs=[])
                        n += 1
                        nop.engine = ins.engine
                        nop.sync_info = mybir.SyncInfo(on_wait=[w], on_update=[])
                        newlist.append(nop)
                    si.on_wait = [waits[-1]]
                newlist.append(ins)
            bb.instructions = newlist
    return n


BITVEC_OPS = (AL.logical_shift_left, AL.logical_shift_right,
              AL.arith_shift_left, AL.arith_shift_right, AL.bitwise_and,
              AL.bitwise_or, AL.bitwise_xor)


def _fix_bitvec_imms(nc):
    """walrus requires integer ImmVal (matching src/dst dtype) for bitvec
    alu ops; bass lowers python ints as f32 imms in scalar_tensor_tensor."""
    for fn in nc.m.functions:
        for bb in fn.blocks:
            for ins in bb.instructions:
                if not isinstance(ins, mybir.InstTensorScalarPtr):
                    continue
                ops = [getattr(ins, "op0", None), getattr(ins, "op1", None)]
                if not any(o in BITVEC_OPS for o in ops if o is not None):
                    continue
                new_ins = []
                for a in ins.ins:
                    if isinstance(a, mybir.ImmediateValue) and \
                            a.dtype == mybir.dt.float32:
                        a = mybir.ImmediateValue(dtype=mybir.dt.int32,
                                                 value=int(a.value))
                    new_ins.append(a)
                ins.ins = new_ins


def build_nc(rows=ROWS, f=F, num_devices=NCORES, fix_multiwaits=True):
    elems = rows * DIM
    assert elems % (128 * f) == 0
    ntiles = elems // (128 * f)
    assert ntiles % G == 0
    ngroups = ntiles // G
    R = f // 8
    GR = G * R

    nc = bass.Bass("TRN2", num_devices=num_devices, debug=False)
    x = nc.dram_tensor("x", [rows, DIM], F32, kind="ExternalInput")
    y = nc.dram_tensor("y", [rows, DIM], BF16, kind="ExternalOutput")
    xt = x[:].flatten().rearrange("(t p f) -> t p f", p=128, f=f)
    yt = y[:].flatten().rearrange("(t p f) -> t p f", p=128, f=f)

    def dv(ap):  # [p, c, r] view of c-major storage (packed r)
        return ap.rearrange("p (c r) -> p c r", c=8)

    def rv(ap):  # [p, r, c] view of c-major storage (c stride R)
        return ap.rearrange("p (c r) -> p r c", c=8)

    def bcr(ap_2d):  # [128, R] -> [128, 8, R] broadcast over c
        p, r = ap_2d.shape
        return ap_2d.unsqueeze(1).broadcast_to((p, 8, r))

    with TileContext(nc) as tc:
        with tc.tile_pool(name="cst", bufs=1) as cst, \
             tc.tile_pool(name="ioi", bufs=4) as ioi, \
             tc.tile_pool(name="pnd", bufs=2) as pnd, \
             tc.tile_pool(name="spk", bufs=3) as spk, \
             tc.tile_pool(name="sf1", bufs=3) as sf1, \
             tc.tile_pool(name="ssg", bufs=3) as ssg, \
             tc.tile_pool(name="trp", bufs=1) as trp, \
             tc.tile_pool(name="red", bufs=2) as red, \
             tc.tile_pool(name="smt", bufs=1) as smt, \
             tc.tile_pool(name="smo", bufs=3) as smo, \
             tc.tile_pool(name="asm", bufs=2) as asm, \
             tc.tile_pool(name="yvp", bufs=2) as yvp, \
             tc.tile_pool(name="psU", bufs=2, space="PSUM") as psU, \
             tc.tile_pool(name="psV", bufs=2, space="PSUM") as psV, \
             tc.tile_pool(name="psY", bufs=2, space="PSUM") as psY:

            ip = cst.tile([128, 128], I32, tag="ip")
            nc.gpsimd.iota(ip[:], pattern=[[0, 128]], base=0,
                           channel_multiplier=1)
            jp = cst.tile([128, 128], I32, tag="jp")
            nc.gpsimd.iota(jp[:], pattern=[[1, 128]], base=0,
                           channel_multiplier=0)
            ident = cst.tile([128, 128], BF16, tag="ident")
            nc.vector.tensor_tensor(ident[:], ip[:], jp[:], AL.is_equal)

            def phase_in(ctx):
                g = ctx["g"]
                xvs = []
                for t in range(G):
                    xv = ioi.tile([128, f], F32, tag="xv")
                    nc.sync.dma_start(xv[:], xt[g * G + t])
                    xvs.append(xv)
                ctx["xvs"] = xvs

            def phase_a(g):
                ctx = {}
                ctx["g"] = g
                pks = ctx["pks"] = spk.tile([128, G * f], F32, tag="pks", name="pks")
                f1s = ctx["f1s"] = sf1.tile([128, G * f], BF16, tag="f1s", name="f1s")
                sgs = ctx["sgs"] = ssg.tile([128, G * f], BF16, tag="sgs", name="sgs")
                L1s = ctx["L1s"] = red.tile([128, GR], F32, tag="L1s", name="L1s")
                mxs = ctx["mxs"] = red.tile([128, GR], F32, tag="mxs", name="mxs")
                mns = ctx["mns"] = red.tile([128, GR], F32, tag="mns", name="mns")
                Uf = ctx["Uf"] = psU.tile([128, GR], F32, tag="Uf", name="Uf")
                Us = ctx["Us"] = psV.tile([128, GR], F32, tag="Us", name="Us")

                return ctx

            def phase_a_tile(ctx, t):
                    g = ctx["g"]
                    pks, f1s, sgs = ctx["pks"], ctx["f1s"], ctx["sgs"]
                    L1s, mxs, mns = ctx["L1s"], ctx["mxs"], ctx["mns"]
                    Uf, Us = ctx["Uf"], ctx["Us"]
                    gt = g * G + t
                    ts_ = slice(t * f, (t + 1) * f)
                    rs = slice(t * R, (t + 1) * R)
                    xv = ctx["xvs"][t]
                    xv_cr = xv[:].rearrange("p (r c) -> p c r", c=8)

                    t1 = pnd.tile([128, f], F32, tag="t1")
                    nc.scalar.activation(dv(t1[:]), xv_cr, AF.Copy,
                                         bias=MAGIC)
                    f1b = f1s[:, ts_]
                    nc.scalar.activation(f1b, t1[:], AF.Copy, bias=-MAGIC)

                    # d1 = x - f1 (c-major storage)
                    d1 = pnd.tile([128, f], F32, tag="d1")
                    nc.gpsimd.tensor_tensor(dv(d1[:]), xv_cr, dv(f1b),
                                            AL.subtract)

                    sgb = sgs[:, ts_]
                    nc.scalar.activation(sgb, d1[:], AF.Sign)

                    pk = pks[:, ts_]
                    nc.vector.tensor_scalar(pk.bitcast(I32),
                                            d1[:].bitcast(I32),
                                            0x7fffffff, None, AL.bitwise_and)

                    # mx/mn: abs-reduces over the component axis
                    nc.vector.tensor_reduce(mxs[:, rs], rv(d1[:]), AX.X,
                                            AL.max,
                                            apply_absolute_value=True)
                    nc.vector.tensor_reduce(mns[:, rs], rv(d1[:]), AX.X,
                                            AL.min,
                                            apply_absolute_value=True)

                    # L1 via Pool add-tree on pk = |d|
                    pk3 = dv(pk)
                    s1 = trp.tile([128, f // 2], F32, tag="s1")
                    s1v = s1[:].rearrange("p (c r) -> p c r", c=4)
                    nc.gpsimd.tensor_tensor(s1v, pk3[:, 0:4], pk3[:, 4:8],
                                            AL.add)
                    s2 = trp.tile([128, f // 4], F32, tag="s2")
                    s2v = s2[:].rearrange("p (c r) -> p c r", c=2)
                    nc.gpsimd.tensor_tensor(s2v, s1v[:, 0:2], s1v[:, 2:4],
                                            AL.add)
                    nc.gpsimd.tensor_tensor(L1s[:, rs], s2v[:, 0],
                                            s2v[:, 1], AL.add)

                    for c in range(8):
                        nc.tensor.matmul(Uf[:, rs], ident[:],
                                         f1b[:, c * R:(c + 1) * R],
                                         start=(c == 0), stop=(c == 7))
                    for c in range(8):
                        nc.tensor.matmul(Us[:, rs], ident[:],
                                         sgb[:, c * R:(c + 1) * R],
                                         start=(c == 0), stop=(c == 7))

            def phase_b(ctx):
                Uf, Us = ctx["Uf"], ctx["Us"]
                L1s, mxs, mns = ctx["L1s"], ctx["mxs"], ctx["mns"]
                fM = smt.tile([128, GR], F32, tag="fM")
                nc.scalar.activation(fM[:], Uf[:], AF.Copy, bias=MAGIC)
                ngM = smt.tile([128, GR], F32, tag="ngM")
                nc.scalar.activation(ngM[:], Us[:], AF.Copy,
                                     bias=MAGIC + 4.0, scale=-0.5)
                po1i = smt.tile([128, GR], I32, tag="po1i")
                nc.vector.tensor_scalar(po1i[:], fM[:].bitcast(I32), 1, None,
                                        AL.bitwise_and)
                pnni = smt.tile([128, GR], I32, tag="pnni")
                nc.vector.tensor_scalar(pnni[:], ngM[:].bitcast(I32), 1, None,
                                        AL.bitwise_and)
                po1b = smt.tile([128, GR], BF16, tag="po1b")
                nc.vector.tensor_copy(po1b[:], po1i[:])
                pnnb = smt.tile([128, GR], BF16, tag="pnnb")
                nc.vector.tensor_copy(pnnb[:], pnni[:])
                po2b = smt.tile([128, GR], BF16, tag="po2b")
                nc.vector.tensor_tensor(po2b[:], po1b[:], pnnb[:],
                                        AL.not_equal)
                tm1 = smt.tile([128, GR], F32, tag="tm1")
                nc.vector.tensor_scalar(tm1[:], mxs[:], -2.0, 1.0,
                                        AL.mult, AL.add)
                tm2 = smt.tile([128, GR], F32, tag="tm2")
                nc.vector.tensor_scalar(tm2[:], mns[:], 2.0, None, AL.mult)
                D1s = smt.tile([128, GR], F32, tag="D1s")
                nc.vector.tensor_scalar(D1s[:], L1s[:], -1.0, 2.0,
                                        AL.mult, AL.add)
                A2s = smt.tile([128, GR], F32, tag="A2s")
                nc.gpsimd.tensor_tensor(A2s[:], po1b[:], tm1[:], AL.mult)
                A1s = smt.tile([128, GR], F32, tag="A1s")
                nc.gpsimd.tensor_tensor(A1s[:], po2b[:], tm2[:], AL.mult)
                D2s = smt.tile([128, GR], F32, tag="D2s")
                nc.gpsimd.tensor_tensor(D2s[:], D1s[:], A1s[:], AL.add)
                c01 = smt.tile([128, GR], F32, tag="c01")
                nc.vector.tensor_tensor(c01[:], D2s[:], A2s[:], AL.is_lt)
                nc01 = smt.tile([128, GR], F32, tag="nc01")
                nc.vector.tensor_scalar(nc01[:], c01[:], -1.0, 1.0,
                                        AL.mult, AL.add)
                # mcc = mn*c01 + mx*(1-c01): exact (one addend always 0)
                mA = smt.tile([128, GR], F32, tag="mA")
                nc.gpsimd.tensor_tensor(mA[:], mns[:], c01[:], AL.mult)
                mB = smt.tile([128, GR], F32, tag="mB")
                nc.gpsimd.tensor_tensor(mB[:], mxs[:], nc01[:], AL.mult)
                mcc = smo.tile([128, GR], F32, tag="mcc")
                nc.gpsimd.tensor_tensor(mcc[:], mA[:], mB[:], AL.add)
                v1s = smt.tile([128, GR], BF16, tag="v1s")
                nc.vector.tensor_tensor(v1s[:], po1b[:], po2b[:], AL.add)
                w2b = smt.tile([128, GR], BF16, tag="w2b")
                nc.gpsimd.tensor_tensor(w2b[:], c01[:], v1s[:], AL.mult)
                gq = smo.tile([128, GR], BF16, tag="gq")
                nc.gpsimd.tensor_tensor(gq[:], po1b[:], w2b[:], AL.subtract)
                ch = smo.tile([128, GR], BF16, tag="ch")
                nc.vector.tensor_scalar(ch[:], c01[:], 0.5, None, AL.mult)
                ctx["mcc"], ctx["gq"], ctx["ch"] = mcc, gq, ch

            def phase_c_tile(ctx, t):
                    pks, f1s, sgs = ctx["pks"], ctx["f1s"], ctx["sgs"]
                    mcc, gq, ch = ctx["mcc"], ctx["gq"], ctx["ch"]
                    gt = ctx["g"] * G + t
                    ts_ = slice(t * f, (t + 1) * f)
                    rs = slice(t * R, (t + 1) * R)
                    oh = ctx["ohs"][t]
                    Av = asm.tile([128, f], BF16, tag="Av")
                    nc.gpsimd.tensor_tensor(dv(Av[:]), dv(oh[:]),
                                            bcr(gq[:, rs]), AL.mult)
                    nc.gpsimd.tensor_tensor(dv(Av[:]), dv(Av[:]),
                                            bcr(ch[:, rs]), AL.add)
                    Cv = Av
                    nc.vector.tensor_tensor(Cv[:], Cv[:], sgs[:, ts_],
                                            AL.mult)
                    yp = psY.tile([128, f], F32, tag="yp")
                    f1iv = rv(f1s[:, ts_])
                    cviv = rv(Cv[:])
                    h = R // 2
                    for hh in range(2):
                        msl = slice(hh * h, (hh + 1) * h)
                        osl = slice(hh * (f // 2), (hh + 1) * (f // 2))
                        nc.tensor.matmul(yp[:, osl], ident[:], f1iv[:, msl],
                                         start=True, stop=False)
                        nc.tensor.matmul(yp[:, osl], ident[:], cviv[:, msl],
                                         start=False, stop=True)
                    yv = yvp.tile([128, f], BF16, tag="yv")
                    nc.scalar.activation(yv[:], yp[:], AF.Copy)
                    nc.sync.dma_start(yt[gt], yv[:])

            # software pipeline: B/C of group g-1 run alongside A of group g
            p1 = None  # group awaiting B
            p2 = None  # group awaiting C
            nxt = phase_a(0)
            phase_in(nxt)

            def phase_c_oh(ctx, t):
                    pks = ctx["pks"]
                    mcc = ctx["mcc"]
                    ts_ = slice(t * f, (t + 1) * f)
                    rs = slice(t * R, (t + 1) * R)
                    pkf3 = dv(pks[:, ts_])
                    dfv = pnd.tile([128, f], F32, tag="t1")
                    nc.gpsimd.tensor_tensor(dv(dfv[:]), pkf3,
                                            bcr(mcc[:, rs]), AL.subtract)
                    oh = asm.tile([128, f], BF16, tag="oh", bufs=2)
                    nc.vector.tensor_scalar(oh[:], dfv[:], 0.0, None,
                                            AL.is_equal)
                    ctx.setdefault("ohs", []).append(oh)

            def do_b(ctx):
                phase_b(ctx)
                for t in range(G):
                    phase_c_oh(ctx, t)

            def do_c(ctx):
                for t in range(G):
                    phase_c_tile(ctx, t)

            for g in range(ngroups):
                cur = nxt
                if g + 1 < ngroups:
                    nxt = phase_a(g + 1)
                    phase_in(nxt)
                for t in range(G):
                    phase_a_tile(cur, t)
                if p1 is not None:
                    do_b(p1)
                if p2 is not None:
                    do_c(p2)
                p2 = p1
                p1 = cur
            do_b(p1)
            do_c(p2)
            do_c(p1)

    _fix_bitvec_imms(nc)
    if fix_multiwaits:
        _split_multiwaits(nc)
    return nc


def reference_numpy_v8(x):
    """Bit-exact numpy mirror of the v13 kernel (row-logical space)."""
    x = np.asarray(x, dtype=np.float32)
    M = np.float32(MAGIC)
    t1 = (x + M).astype(np.float32)
    f1 = (t1 - M).astype(np.float32)
    d1 = (x - f1).astype(np.float32)
    sg = np.sign(d1).astype(np.float32)
    pkf = np.abs(d1).astype(np.float32)
    # L1 pair tree: ((a0+a4)+(a2+a6)) style
    s1 = (pkf[:, 0:4] + pkf[:, 4:8]).astype(np.float32)
    s2 = (s1[:, 0:2] + s1[:, 2:4]).astype(np.float32)
    L1 = (s2[:, 0] + s2[:, 1]).astype(np.float32)
    mx = pkf.max(axis=1)
    mn = pkf.min(axis=1)
    Uf = f1.sum(axis=1, dtype=np.float32)
    Us = sg.sum(axis=1, dtype=np.float32)
    fM = (Uf + M).astype(np.float32)
    ngM = (np.float32(-0.5) * Us + np.float32(MAGIC + 4.0)).astype(np.float32)
    po1 = (fM.view(np.int32) & 1).astype(np.float32)
    pnn = (ngM.view(np.int32) & 1).astype(np.float32)
    po2 = (po1 != pnn).astype(np.float32)
    tm1 = (np.float32(-2.0) * mx + np.float32(1.0)).astype(np.float32)
    tm2 = (np.float32(2.0) * mn).astype(np.float32)
    D1s = (np.float32(-1.0) * L1 + np.float32(2.0)).astype(np.float32)
    A2s = (po1 * tm1).astype(np.float32)
    A1s = (po2 * tm2).astype(np.float32)
    D2s = (D1s + A1s).astype(np.float32)
    c01 = (D2s < A2s).astype(np.float32)
    mcc = np.where(c01 > 0, mn, mx)  # exact mult-select
    v1s = (po1 + po2).astype(np.float32)
    w2 = (c01 * v1s).astype(np.float32)
    gq = (po1 - w2).astype(np.float32)
    ch = (c01 * np.float32(0.5)).astype(np.float32)
    oh = (pkf == mcc[:, None]).astype(np.float32)
    Av = oh * gq[:, None]
    Bv = Av + ch[:, None]
    Cv = (sg * Bv).astype(np.float32)
    yv = (f1 + Cv).astype(np.float32)
    return yv


reference_numpy_v7 = reference_numpy_v8
reference_numpy_v6 = reference_numpy_v8


_NC_CACHE = {}


def _get_nc(rows, f):
    key = (rows, f)
    if key not in _NC_CACHE:
        _NC_CACHE[key] = build_nc(rows, f)
    return _NC_CACHE[key]


def kernel(x: np.ndarray, _trace=False) -> np.ndarray:
    from concourse.bass_utils import run_bass_kernel_spmd
    assert x.shape == (N_ROWS_FULL, DIM), x.shape
    x = np.ascontiguousarray(np.asarray(x, dtype=np.float32))
    nc = _get_nc(ROWS, F)
    in_maps = [
        {"x": np.ascontiguousarray(x[i * ROWS:(i + 1) * ROWS])}
        for i in range(NCORES)
    ]
    res = run_bass_kernel_spmd(nc, in_maps, core_ids=list(range(NCORES)),
                               trace=_trace)
    out = np.empty_like(x)
    for i in range(NCORES):
        out[i * ROWS:(i + 1) * ROWS] = \
            np.asarray(res.results[i]["y"]).astype(np.float32)
    return out
